# revision 4
# baseline (speedup 1.0000x reference)
"""Trainium2 Bass kernel for MultiHeadedAttention (B=2, L=2048, D=1024, H=16).

Sharding: 8 cores = 2 batches x 4 head-groups. Core c handles batch c//4,
heads 4*(c%4)..4*(c%4)+3; its attention (projections, scores, softmax, P@V)
is fully local. The residual + LayerNorm needs full d_model rows, obtained
with an on-device AllGather over each 4-core batch group; LN is computed
redundantly per core and the host keeps one copy per batch.

kernel(**inputs) takes the full unsharded inputs and returns
(normed [2,2048,1024] f32, attn [32,2048,2048] f32) matching reference().
"""
import sys

sys.path.insert(0, "/opt/trn_rl_repo")

import numpy as np
import concourse.bass as bass  # noqa: F401
import concourse.tile as tile
from concourse import bacc, mybir
from concourse import bass_utils
from concourse.masks import make_identity

f32 = mybir.dt.float32
f32r = mybir.dt.float32r
bf16 = mybir.dt.bfloat16
u8 = mybir.dt.uint8
AF = mybir.ActivationFunctionType
ALU = mybir.AluOpType

N_CORES = 8
EPS = 1e-6


def build_mha_nc(L=2048, D=1024, HL=4, DK=64, n_cores=N_CORES, group=4):
    """One-core SPMD program: attention for HL local heads over [L, D]."""
    P = 128
    RG = min(512, L)                 # rows per rowgroup
    NG = L // RG                     # rowgroups
    RT_PER_G = RG // P               # rowtiles per rowgroup
    NT = L // P                      # total rowtiles
    ND = D // P                      # d chunks (contraction)
    K4 = min(512, L)                 # wide key chunk
    NK4 = L // K4
    NKC = L // P                     # 128-wide key chunks
    DH = HL * DK                     # per-core projected width
    NCH = max(1, DH // P)            # col chunks of projected width
    CW = min(DH, P)                  # col chunk width
    assert DH % CW == 0 and D % P == 0 and L % P == 0

    nc = bacc.Bacc("TRN2", target_bir_lowering=False, debug=False,
                   num_devices=n_cores)

    xk = nc.dram_tensor("xk", [L, D], f32, kind="ExternalInput").ap()
    xv = nc.dram_tensor("xv", [L, D], f32, kind="ExternalInput").ap()
    xq = nc.dram_tensor("xq", [L, D], f32, kind="ExternalInput").ap()
    msk = nc.dram_tensor("msk", [L, L], u8, kind="ExternalInput").ap()
    wk = nc.dram_tensor("wk", [D, DH], f32, kind="ExternalInput").ap()
    wv = nc.dram_tensor("wv", [D, DH], f32, kind="ExternalInput").ap()
    wq = nc.dram_tensor("wq", [D, DH], f32, kind="ExternalInput").ap()
    g_in = nc.dram_tensor("g_in", [D], f32, kind="ExternalInput").ap()
    b_in = nc.dram_tensor("b_in", [D], f32, kind="ExternalInput").ap()

    attn_o = nc.dram_tensor("attn_o", [HL, L, L], f32,
                            kind="ExternalOutput").ap()
    norm_o = nc.dram_tensor("norm_o", [L, D], f32, kind="ExternalOutput").ap()

    scale = 1.0 / float(np.sqrt(DK))

    with tile.TileContext(nc) as tc:
        with tc.tile_pool(name="persist", bufs=1) as persist, \
             tc.tile_pool(name="drp", bufs=1, space="DRAM") as drp:
            out_part = drp.tile([L, DH], f32)
            gath = drp.tile([group, L, DH], f32)

            ident = persist.tile([P, P], f32)
            make_identity(nc, ident[:])
            ident_bf = persist.tile([P, P], bf16)
            nc.vector.tensor_copy(ident_bf[:], ident[:])

            # weights as lhsT chunks: [128 d, ND*DH free], slice [:, dc*DH+c]
            wq_sb = persist.tile([P, ND * DH], f32r)
            wk_sb = persist.tile([P, ND * DH], f32r)
            wv_sb = persist.tile([P, ND * DH], f32r)
            for w_sb, w in ((wq_sb, wq), (wk_sb, wk), (wv_sb, wv)):
                for dc in range(ND):
                    nc.gpsimd.dma_start(w_sb[:, dc * DH:(dc + 1) * DH],
                                        w[dc * P:(dc + 1) * P, :])

            # projected tensors, persist across phases
            qT_sb = [persist.tile([CW, L], f32r, name=f"qT{i}")
                     for i in range(NCH)]
            kT_sb = [persist.tile([CW, L], f32r, name=f"kT{i}")
                     for i in range(NCH)]
            v_sb = [persist.tile([P, NKC * DK], bf16, name=f"v{h}")
                    for h in range(HL)]

            # ---------------- Phase A: projections ----------------
            with tc.tile_pool(name="xload", bufs=4) as xload, \
                 tc.tile_pool(name="xtst", bufs=1) as xtst, \
                 tc.tile_pool(name="tps", bufs=4, space="PSUM") as tps, \
                 tc.tile_pool(name="pps", bufs=2, space="PSUM") as pps, \
                 tc.tile_pool(name="vps", bufs=2, space="PSUM") as vps:
                for rg in range(NG):
                    xTs = {}
                    for nm, xin in (("k", xk), ("v", xv), ("q", xq)):
                        xT = xtst.tile([P, ND * RG], f32r, name=f"xT{nm}",
                                       tag=f"xT{nm}")
                        xTs[nm] = xT
                        for rt in range(RT_PER_G):
                            r0 = rg * RG + rt * P
                            xrow = xload.tile([P, D], f32, tag="xrow")
                            nc.sync.dma_start(xrow[:], xin[r0:r0 + P, :])
                            for dc in range(ND):
                                tp = tps.tile([P, P], f32, tag="tp")
                                nc.tensor.transpose(
                                    tp[:], xrow[:, dc * P:(dc + 1) * P],
                                    ident[:])
                                dst = xT[:, dc * RG + rt * P:
                                         dc * RG + rt * P + P]
                                if dc % 2 == 0:
                                    nc.scalar.copy(dst, tp[:])
                                else:
                                    nc.vector.tensor_copy(dst, tp[:])
                    # q,k projections in y^T orientation: out [col, rows]
                    for w_sb, dstT, xkey in ((wq_sb, qT_sb, "q"),
                                             (wk_sb, kT_sb, "k")):
                        xin = xTs[xkey]
                        for ch in range(NCH):
                            ps = pps.tile([CW, RG], f32, tag="pps")
                            for dc in range(ND):
                                nc.tensor.matmul(
                                    ps[:],
                                    w_sb[:, dc * DH + ch * CW:
                                         dc * DH + (ch + 1) * CW],
                                    xin[:, dc * RG:(dc + 1) * RG],
                                    start=(dc == 0), stop=(dc == ND - 1))
                            nc.vector.tensor_copy(
                                dstT[ch][:, rg * RG:(rg + 1) * RG], ps[:])
                    # v projection, natural orientation: out [rows, cols]
                    for rt in range(RT_PER_G):
                        ps = vps.tile([P, DH], f32, tag="vps")
                        for dc in range(ND):
                            nc.tensor.matmul(
                                ps[:],
                                xTs["v"][:, dc * RG + rt * P:
                                         dc * RG + rt * P + P],
                                wv_sb[:, dc * DH:(dc + 1) * DH],
                                start=(dc == 0), stop=(dc == ND - 1))
                        kc = rg * RT_PER_G + rt
                        for h in range(HL):
                            nc.scalar.copy(
                                v_sb[h][:, kc * DK:(kc + 1) * DK],
                                ps[:, h * DK:(h + 1) * DK])

            # ---------------- Phase B: attention ----------------
            with tc.tile_pool(name="mload", bufs=2) as mload, \
                 tc.tile_pool(name="m01p", bufs=2) as m01p, \
                 tc.tile_pool(name="expp", bufs=2) as expp, \
                 tc.tile_pool(name="attnp", bufs=2) as attnp, \
                 tc.tile_pool(name="pbfp", bufs=2) as pbfp, \
                 tc.tile_pool(name="ptp", bufs=1) as ptp, \
                 tc.tile_pool(name="avtp", bufs=2) as avtp, \
                 tc.tile_pool(name="outst", bufs=2) as outst, \
                 tc.tile_pool(name="smalls", bufs=10) as smalls, \
                 tc.tile_pool(name="spsp", bufs=3, space="PSUM") as spsp, \
                 tc.tile_pool(name="ptpsp", bufs=2, space="PSUM") as ptpsp, \
                 tc.tile_pool(name="avpsp", bufs=2, space="PSUM") as avpsp, \
                 tc.tile_pool(name="rtpsp", bufs=1, space="PSUM") as rtpsp:
                for rg in range(NG):
                    mrows = []
                    for rt in range(RT_PER_G):
                        r0 = rg * RG + rt * P
                        mrow = mload.tile([P, L], u8, tag=f"mrow{rt}")
                        nc.sync.dma_start(mrow[:], msk[r0:r0 + P, :])
                        mrows.append(mrow)
                    out_stage = [outst.tile([P, DH], f32, tag=f"ost{rt}", name=f"ost{rt}")
                                 for rt in range(RT_PER_G)]
                    for h in range(HL):
                        ch = (h * DK) // CW
                        po = (h * DK) % CW
                        pt_stage = ptp.tile([P, NKC * RG], bf16, tag="pt")
                        recips = []
                        for rt in range(RT_PER_G):
                            r0 = rg * RG + rt * P
                            m01 = m01p.tile([P, L], f32, tag="m01")
                            nc.gpsimd.tensor_scalar(
                                m01[:], mrows[rt][:], -1.0, 1.0,
                                ALU.mult, ALU.add)
                            exp_raw = expp.tile([P, L], f32, tag="exp")
                            for k4 in range(NK4):
                                spsum = spsp.tile([P, K4], f32, tag="sps")
                                nc.tensor.matmul(
                                    spsum[:],
                                    qT_sb[ch][po:po + DK, r0:r0 + P],
                                    kT_sb[ch][po:po + DK,
                                              k4 * K4:(k4 + 1) * K4],
                                    start=True, stop=True)
                                nc.scalar.activation(
                                    exp_raw[:, k4 * K4:(k4 + 1) * K4],
                                    spsum[:], AF.Exp, scale=scale)
                            # masked P (bf16) + row sums
                            p_bf = pbfp.tile([P, L], bf16, tag="pbf")
                            sums = smalls.tile([P, 1], f32, tag="sums")
                            nc.vector.scalar_tensor_tensor(
                                p_bf[:], exp_raw[:], 1.0, m01[:],
                                ALU.mult, ALU.mult, accum_out=sums[:])
                            recip = smalls.tile([P, 1], f32, tag="recip",
                                                bufs=2 * RT_PER_G)
                            nc.vector.reciprocal(recip[:], sums[:])
                            recips.append(recip)
                            # attn (f32) = exp * recip * mask01 -> HBM
                            attn_t = attnp.tile([P, L], f32, tag="attn")
                            nc.vector.scalar_tensor_tensor(
                                attn_t[:], exp_raw[:], recip[:], m01[:],
                                ALU.mult, ALU.mult)
                            nc.sync.dma_start(attn_o[h, r0:r0 + P, :],
                                              attn_t[:])
                            # transpose P -> pt_stage [k-chunk-part, kc*RG+q]
                            for k4 in range(NK4):
                                nj = K4 // P
                                pt_ps = ptpsp.tile([P, K4], bf16, tag="ptps")
                                for j in range(nj):
                                    kc = k4 * nj + j
                                    nc.tensor.transpose(
                                        pt_ps[:, j * P:(j + 1) * P],
                                        p_bf[:, kc * P:(kc + 1) * P],
                                        ident_bf[:])
                                src = pt_ps[:].rearrange(
                                    "p (j q) -> p j q", q=P)
                                dst = pt_stage[:].rearrange(
                                    "p (kc q) -> p kc q", q=RG)[
                                    :, k4 * nj:(k4 + 1) * nj,
                                    rt * P:rt * P + P]
                                if (rt + k4) % 2 == 0:
                                    nc.scalar.copy(dst, src)
                                else:
                                    nc.vector.tensor_copy(dst, src)
                        # av for the whole rowgroup: out^T [DK, RG]
                        av_ps = avpsp.tile([DK, RG], f32, tag="avps")
                        for kc in range(NKC):
                            nc.tensor.matmul(
                                av_ps[:],
                                v_sb[h][:, kc * DK:(kc + 1) * DK],
                                pt_stage[:, kc * RG:(kc + 1) * RG],
                                start=(kc == 0), stop=(kc == NKC - 1))
                        avT = avtp.tile([DK, RG], f32, tag="avT")
                        nc.vector.tensor_copy(avT[:], av_ps[:])
                        # retranspose per rowtile, normalize by row sums
                        for rt in range(RT_PER_G):
                            rt_ps = rtpsp.tile([P, DK], f32, tag="rtps")
                            nc.tensor.transpose(
                                rt_ps[:], avT[:, rt * P:(rt + 1) * P],
                                ident[:DK, :DK])
                            nc.vector.tensor_scalar_mul(
                                out_stage[rt][:, h * DK:(h + 1) * DK],
                                rt_ps[:], recips[rt][:])
                    for rt in range(RT_PER_G):
                        r0 = rg * RG + rt * P
                        nc.sync.dma_start(out_part[r0:r0 + P, :],
                                          out_stage[rt][:])

            # ---------- Phase C: AllGather + residual + LayerNorm ----------
            with tc.tile_pool(name="lnp", bufs=3) as lnp, \
                 tc.tile_pool(name="lnw", bufs=1) as lnw, \
                 tc.tile_pool(name="lns", bufs=6) as lns:
                groups = [list(range(i, i + group))
                          for i in range(0, n_cores, group)]
                nc.gpsimd.collective_compute(
                    "AllGather", ALU.bypass, replica_groups=groups,
                    ins=[out_part[:].opt()], outs=[gath[:].opt()])
                g_bc = lnw.tile([P, D], f32)
                nc.sync.dma_start(g_bc[:], g_in[None, :].to_broadcast((P, D)))
                b_bc = lnw.tile([P, D], f32)
                nc.sync.dma_start(b_bc[:], b_in[None, :].to_broadcast((P, D)))
                eps_t = lnw.tile([P, 1], f32)
                nc.vector.memset(eps_t[:], EPS)
                for lt in range(NT):
                    r0 = lt * P
                    out_t = lnp.tile([P, D], f32, tag="out_t")
                    for j in range(group):
                        nc.sync.dma_start(out_t[:, j * DH:(j + 1) * DH],
                                          gath[j, r0:r0 + P, :])
                    qrow = lnp.tile([P, D], f32, tag="qrow")
                    nc.sync.dma_start(qrow[:], xq[r0:r0 + P, :])
                    res_t = lnp.tile([P, D], f32, tag="res_t")
                    nc.vector.tensor_add(res_t[:], out_t[:], qrow[:])
                    mu_neg = lns.tile([P, 1], f32, tag="mu")
                    nc.vector.reduce_sum(mu_neg[:], res_t[:],
                                         axis=mybir.AxisListType.X)
                    nc.scalar.mul(mu_neg[:], mu_neg[:], -1.0 / D)
                    sq_t = lnp.tile([P, D], f32, tag="sq_t")
                    ssq = lns.tile([P, 1], f32, tag="ssq")
                    nc.scalar.activation(sq_t[:], res_t[:], AF.Square,
                                         bias=mu_neg[:], accum_out=ssq[:])
                    istd = lns.tile([P, 1], f32, tag="istd")
                    nc.scalar.activation(istd[:], ssq[:], AF.Sqrt,
                                         scale=1.0 / D, bias=eps_t[:])
                    nc.vector.reciprocal(istd[:], istd[:])
                    t1 = lnp.tile([P, D], f32, tag="t1")
                    nc.vector.tensor_scalar(t1[:], res_t[:], mu_neg[:],
                                            istd[:], ALU.add, ALU.mult)
                    t2 = lnp.tile([P, D], f32, tag="t2")
                    nc.vector.tensor_mul(t2[:], t1[:], g_bc[:])
                    nout = lnp.tile([P, D], f32, tag="nout")
                    nc.vector.tensor_add(nout[:], t2[:], b_bc[:])
                    nc.sync.dma_start(norm_o[r0:r0 + P, :], nout[:])

    nc.compile()
    return nc


_NC_CACHE = {}


def _get_nc():
    if "nc" not in _NC_CACHE:
        _NC_CACHE["nc"] = build_mha_nc()
    return _NC_CACHE["nc"]


def shard_inputs(key, value, query, mask, W_k, W_v, W_q, ln_g, ln_b,
                 n_cores=N_CORES, H=16):
    B, L, D = key.shape
    DK = D // H
    group = n_cores // B
    HL = H // group
    DH = HL * DK
    key = np.ascontiguousarray(np.asarray(key, dtype=np.float32))
    value = np.ascontiguousarray(np.asarray(value, dtype=np.float32))
    query = np.ascontiguousarray(np.asarray(query, dtype=np.float32))
    mask_u8 = np.ascontiguousarray(np.asarray(mask).astype(np.uint8))
    W_k = np.asarray(W_k, dtype=np.float32)
    W_v = np.asarray(W_v, dtype=np.float32)
    W_q = np.asarray(W_q, dtype=np.float32)
    ln_g = np.ascontiguousarray(np.asarray(ln_g, dtype=np.float32))
    ln_b = np.ascontiguousarray(np.asarray(ln_b, dtype=np.float32))
    in_maps = []
    for c in range(n_cores):
        b = c // group
        hp = c % group
        cs = slice(hp * DH, (hp + 1) * DH)
        in_maps.append({
            "xk": key[b], "xv": value[b], "xq": query[b],
            "msk": mask_u8[b],
            "wk": np.ascontiguousarray(W_k[:, cs]),
            "wv": np.ascontiguousarray(W_v[:, cs]),
            "wq": np.ascontiguousarray(W_q[:, cs]),
            "g_in": ln_g, "b_in": ln_b,
        })
    return in_maps


def assemble_outputs(results, B=2, L=2048, D=1024, H=16, n_cores=N_CORES):
    group = n_cores // B
    HL = H // group
    attn = np.empty((B * H, L, L), dtype=np.float32)
    for c in range(n_cores):
        b = c // group
        hp = c % group
        h0 = b * H + hp * HL
        attn[h0:h0 + HL] = results[c]["attn_o"]
    normed = np.stack([results[b * group]["norm_o"] for b in range(B)])
    return normed, attn


def kernel(key, value, query, mask, W_k, W_v, W_q, ln_g, ln_b):
    B, L, D = np.asarray(key).shape
    H = 16
    nc = _get_nc()
    in_maps = shard_inputs(key, value, query, mask, W_k, W_v, W_q,
                           ln_g, ln_b)
    res = bass_utils.run_bass_kernel_spmd(
        nc, in_maps, core_ids=list(range(N_CORES)))
    return assemble_outputs(res.results, B=B, L=L, D=D, H=H)
